# revision 13
# baseline (speedup 1.0000x reference)
"""CrossAttention TRN2 kernel v2: 8-core SPMD, shard = (batch, S1-half).

Host pre-transposes q,k,v (-> qT/kT/vT chunk arrays) and pre-packs weights, so
the device does zero layout transposes. Per core:
  1. Load qT/kT/vT (f32r, rounded at input), packed Wq/Wk/Wv (f32r), Wo (bf16,
     per-head [p, h*512+d]).
  2. Projections: qhT[h] [65,1024] / khT[h] [65,2048] (row 64 = negm / ones),
     vh_aug[ti] [128, 8*65] bf16 (per-head 64 cols + ones col -> fused PV+rowsum).
  3. Per head: raw-S max pass ([s,t] psum -> DVE negated rowmax -> gpsimd
     min/cast -> per-qi DMA into qhT row 64), S^T pass with K=65 (ones row in
     khT adds -max[s]), ACT exp(scale=1/8) -> P^T bf16, PV single matmul M=65
     accumulating oT[65,512] x2 (row 64 = softmax denominator).
     Pipelined emission: S^T(ti) | maxpass(h+1) piece | exp(ti) | PV(ti-1);
     S^T and maxpass share one 3-buf [128,1024] PSUM pool.
  4. Tail per head (spread into next head's loop): oT->SBUF (ACT), den row ->
     partition 0 via tiny DMA, DVE reciprocal, gpsimd broadcast+multiply ->
     normalized oT_hn[h] [64,1024] bf16.
  5. Final proj out[s,d] = sum_h oT_hn[h]^T @ Wo_h (K=64 per head, psum
     accumulate over heads) -> 8 store DMAs. No transposes anywhere.
"""
import sys
import functools

sys.path.insert(0, "/opt/trn_rl_repo")
import numpy as np
from contextlib import ExitStack

B, S1, S2, D, H, P = 4, 2048, 2048, 512, 8, 64
SC = S1 // 2          # 1024 q rows per core
NCORES = 8
DCH = D // 128        # 4 d-chunks
QT = SC // 128        # 8 q s-tiles
TT = S2 // 128        # 16 t-tiles


@functools.lru_cache(maxsize=1)
def _build():
    from concourse import bacc, tile, mybir

    f32 = mybir.dt.float32
    f32r = mybir.dt.float32r
    bf16 = mybir.dt.bfloat16

    nc = bacc.Bacc("TRN2", target_bir_lowering=False, debug=False)

    qT_d = nc.dram_tensor("qT", [DCH, 128, SC], f32r, kind="ExternalInput").ap()
    kT_d = nc.dram_tensor("kT", [DCH, 128, S2], f32r, kind="ExternalInput").ap()
    vT_d = nc.dram_tensor("vT", [DCH, 128, S2], f32r, kind="ExternalInput").ap()
    wq_d = nc.dram_tensor("wq", [DCH, 128, H * P], f32r, kind="ExternalInput").ap()
    wk_d = nc.dram_tensor("wk", [DCH, 128, H * P], f32r, kind="ExternalInput").ap()
    wv_d = nc.dram_tensor("wv", [DCH, 128, H * P], f32r, kind="ExternalInput").ap()
    wo_d = nc.dram_tensor("wo", [P, H * D], bf16, kind="ExternalInput").ap()
    out_d = nc.dram_tensor("out", [SC, D], f32, kind="ExternalOutput").ap()

    with tile.TileContext(nc) as tc, ExitStack() as ctx:
        # ---- persistent SBUF (allocate all tags up front) ----
        acts = ctx.enter_context(tc.tile_pool(name="acts", bufs=1))
        qhT = [acts.tile([65, SC], f32r, tag=f"qhT{h}", name=f"qhT{h}") for h in range(H)]
        khT = [acts.tile([65, S2], f32r, tag=f"khT{h}", name=f"khT{h}") for h in range(H)]
        vh = [acts.tile([128, H * 65], bf16, tag=f"vh{t}", name=f"vh{t}") for t in range(TT)]
        wo_sb = acts.tile([P, H * D], bf16, tag="wo", name="wo_sb")
        oT_hn = [acts.tile([P, SC], bf16, tag=f"ohn{h}", name=f"ohn{h}") for h in range(H)]

        small = ctx.enter_context(tc.tile_pool(name="small", bufs=4))

        # shared PSUM work pool: S^T tiles and maxpass tiles (3 x 2 banks)
        work_ps = ctx.enter_context(tc.tile_pool(name="work", bufs=3, space="PSUM"))

        # ---- ones rows of khT (gpsimd, overlapped with loads) ----
        for h in range(H):
            nc.gpsimd.memset(khT[h].bitcast(f32)[64:65, :], 1.0)

        copy_alt = [0]

        def copy_split(dst0, src0, dst1, src1):
            """Two psum->sbuf copies, alternating ACT/DVE."""
            if copy_alt[0] % 2 == 0:
                nc.scalar.copy(dst0, src0)
                nc.vector.tensor_copy(dst1, src1)
            else:
                nc.vector.tensor_copy(dst0, src0)
                nc.scalar.copy(dst1, src1)
            copy_alt[0] += 1

        # ---- maxpass pieces (head h, interleaved into other loops) ----
        nacc = [small.tile([128, 1], f32, tag=f"nacc{q % 2}", name=f"nacc{q}") for q in range(QT)]

        def maxpass_piece(h, qi, half):
            mx = work_ps.tile([128, 1024], f32, tag="work", name=f"mx{h}_{qi}_{half}")
            for tb in range(2):
                nc.tensor.matmul(
                    mx[:, tb * 512:(tb + 1) * 512],
                    qhT[h][0:64, qi * 128:(qi + 1) * 128],
                    khT[h][0:64, half * 1024 + tb * 512: half * 1024 + (tb + 1) * 512],
                    start=True, stop=True,
                )
            negm_p = small.tile([128, 1], f32, tag="negp", name=f"negp{h}_{qi}_{half}")
            nc.vector.tensor_reduce(
                negm_p[:], mx[:], axis=mybir.AxisListType.X,
                op=mybir.AluOpType.max, negate=True,
            )
            if half == 0:
                nc.gpsimd.tensor_copy(nacc[qi][:], negm_p[:])
            else:
                nc.gpsimd.tensor_scalar_min(nacc[qi][:], negm_p[:], nacc[qi][:])
                negm_r = small.tile([128, 1], f32r, tag="negr", name=f"negr{h}_{qi}")
                nc.gpsimd.tensor_copy(negm_r[:], nacc[qi][:])
                nc.sync.dma_start(
                    qhT[h][64:65, qi * 128:(qi + 1) * 128], negm_r[:],
                )

        # ---- setup: loads + projections ----
        with tc.tile_pool(name="qph", bufs=1) as qpool, \
             tc.tile_pool(name="proj_ps", bufs=2, space="PSUM") as proj_ps:
            wq_sb = qpool.tile([128, DCH * H * P], f32r, tag="wq", name="wq_sb")
            qT_sb = qpool.tile([128, DCH * SC], f32r, tag="qT", name="qT_sb")
            for c in range(DCH):
                nc.sync.dma_start(wq_sb[:, c * 512:(c + 1) * 512], wq_d[c])
            for c in range(DCH):
                nc.sync.dma_start(qT_sb[:, c * SC:(c + 1) * SC], qT_d[c])
            for hp in range(H // 2):
                for sb in range(2):
                    ps = proj_ps.tile([128, 512], f32, tag="pp", name=f"qp{hp}_{sb}")
                    for c in range(DCH):
                        nc.tensor.matmul(
                            ps[:],
                            wq_sb[:, c * 512 + hp * 128: c * 512 + (hp + 1) * 128],
                            qT_sb[:, c * SC + sb * 512: c * SC + sb * 512 + 512],
                            start=(c == 0), stop=(c == DCH - 1),
                        )
                    copy_split(
                        qhT[2 * hp][0:64, sb * 512:(sb + 1) * 512], ps[0:64, :],
                        qhT[2 * hp + 1][0:64, sb * 512:(sb + 1) * 512], ps[64:128, :],
                    )

            with tc.tile_pool(name="kph", bufs=1) as kpool:
                wk_sb = kpool.tile([128, DCH * H * P], f32r, tag="wk", name="wk_sb")
                kT_sb = kpool.tile([128, DCH * S2], f32r, tag="kT", name="kT_sb")
                for c in range(DCH):
                    nc.sync.dma_start(wk_sb[:, c * 512:(c + 1) * 512], wk_d[c])
                for c in range(DCH):
                    nc.sync.dma_start(kT_sb[:, c * S2:(c + 1) * S2], kT_d[c])
                for hp in range(H // 2):
                    for tb in range(4):
                        ps = proj_ps.tile([128, 512], f32, tag="pp", name=f"kp{hp}_{tb}")
                        for c in range(DCH):
                            nc.tensor.matmul(
                                ps[:],
                                wk_sb[:, c * 512 + hp * 128: c * 512 + (hp + 1) * 128],
                                kT_sb[:, c * S2 + tb * 512: c * S2 + tb * 512 + 512],
                                start=(c == 0), stop=(c == DCH - 1),
                            )
                        copy_split(
                            khT[2 * hp][0:64, tb * 512:(tb + 1) * 512], ps[0:64, :],
                            khT[2 * hp + 1][0:64, tb * 512:(tb + 1) * 512], ps[64:128, :],
                        )

            with tc.tile_pool(name="vph", bufs=1) as vpool:
                wv_sb = vpool.tile([128, DCH * H * P], f32r, tag="wv", name="wv_sb")
                vT_sb = vpool.tile([128, DCH * S2], f32r, tag="vT", name="vT_sb")
                for c in range(DCH):
                    nc.sync.dma_start(wv_sb[:, c * 512:(c + 1) * 512], wv_d[c])
                for c in range(DCH):
                    nc.sync.dma_start(vT_sb[:, c * S2:(c + 1) * S2], vT_d[c])
                nc.sync.dma_start(wo_sb[:], wo_d)
                # v-proj with maxpass(0) interleaved (one piece per t-tile)
                for ti in range(TT):
                    ps = proj_ps.tile([128, 512], f32, tag="pp", name=f"vp{ti}")
                    for c in range(DCH):
                        nc.tensor.matmul(
                            ps[:],
                            vT_sb[:, c * S2 + ti * 128: c * S2 + (ti + 1) * 128],
                            wv_sb[:, c * 512:(c + 1) * 512],
                            start=(c == 0), stop=(c == DCH - 1),
                        )
                    vdst = vh[ti][:].rearrange("t (h q) -> t h q", h=H, q=65)
                    eng = nc.scalar if ti % 2 == 0 else nc.vector
                    if ti % 2 == 0:
                        nc.scalar.copy(vdst[:, :, 0:64],
                                       ps[:].rearrange("t (h q) -> t h q", h=H, q=64))
                    else:
                        nc.vector.tensor_copy(vdst[:, :, 0:64],
                                              ps[:].rearrange("t (h q) -> t h q", h=H, q=64))
                    nc.gpsimd.memset(vdst[:, :, 64:65], 1.0)
                    maxpass_piece(0, ti // 2, ti % 2)
                # head 1's qi=0 pieces belong to "ti 14/15 of head -1" = here
                maxpass_piece(1, 0, 0)
                maxpass_piece(1, 0, 1)

        # ---- attention-phase pools (opened after setup pools freed) ----
        tail_pool = ctx.enter_context(tc.tile_pool(name="tail", bufs=1))
        oT_h_t = [tail_pool.tile([65, SC], f32, tag=f"oth{i}", name=f"oth{i}") for i in range(2)]
        dn_t = [tail_pool.tile([1, SC], f32, tag=f"dn{i}", name=f"dn{i}") for i in range(2)]
        rec_t = [tail_pool.tile([1, SC], f32, tag=f"rec{i}", name=f"rec{i}") for i in range(2)]
        recb_t = [tail_pool.tile([P, SC], f32, tag=f"recb{i}", name=f"recb{i}") for i in range(2)]
        pt_pool = ctx.enter_context(tc.tile_pool(name="pt", bufs=3))
        fin_pool = ctx.enter_context(tc.tile_pool(name="fin", bufs=2))

        # ---- attention ----
        oT_tiles = {}

        def pv(h, tj, ptile):
            for sb in range(2):
                nc.tensor.matmul(
                    oT_tiles[h][sb][0:65, :],
                    vh[tj][:, h * 65:(h + 1) * 65],
                    ptile[:, sb * 512:(sb + 1) * 512],
                    start=(tj == 0), stop=(tj == TT - 1),
                )

        def tail_copy(h):
            """oT psum -> SBUF staging (emit right after PV(15) of head h)."""
            i = h % 2
            oT_h, oTs = oT_h_t[i], oT_tiles[h]
            nc.scalar.copy(oT_h[:, 0:512], oTs[0][:])
            nc.scalar.copy(oT_h[:, 512:1024], oTs[1][:])

        def emit_tail(h):
            """Normalize head h's oT_h (spread into next head's loop)."""
            i = h % 2
            oT_h, dn, rec, recb = oT_h_t[i], dn_t[i], rec_t[i], recb_t[i]

            def p1():
                nc.sync.dma_start(dn[0:1, :], oT_h[64:65, :])
            def p2():
                nc.vector.reciprocal(rec[0:1, :], dn[0:1, :])
            def p3():
                nc.gpsimd.partition_broadcast(recb[0:P, :], rec[0:1, :], channels=P)
            def p4():
                nc.gpsimd.tensor_tensor(
                    oT_hn[h][0:P, :], oT_h[0:P, :], recb[0:P, :],
                    op=mybir.AluOpType.mult,
                )
            return [p1, p2, p3, p4]

        with tc.tile_pool(name="oT_ps", bufs=1, space="PSUM") as oT_ps:
            for h in range(H):
                oT_tiles[h] = [
                    oT_ps.tile([65, 512], f32, tag=f"oT{sb}", name=f"oT{h}_{sb}")
                    for sb in range(2)
                ]
                pts = {}
                tail_cl = emit_tail(h - 1) if h > 0 else []
                for ti in range(TT):
                    st = work_ps.tile([128, 1024], f32, tag="work", name=f"st{h}_{ti}")
                    for sb in range(2):
                        nc.tensor.matmul(
                            st[:, sb * 512:(sb + 1) * 512],
                            khT[h][0:65, ti * 128:(ti + 1) * 128],
                            qhT[h][0:65, sb * 512:(sb + 1) * 512],
                            start=True, stop=True,
                        )
                    # maxpass pieces shifted 2 slots early: head h+1's qi=0
                    # was emitted at ti 14/15 of head h-1, so the last negm
                    # DMA (qi=7) issues at ti=13 and its latency hides.
                    if h + 1 < H and ti <= 13:
                        maxpass_piece(h + 1, (ti + 2) // 2, (ti + 2) % 2)
                    if h + 2 < H and ti >= 14:
                        maxpass_piece(h + 2, 0, ti - 14)
                    ptile = pt_pool.tile([128, 1024], bf16, tag="pt", name=f"pt{h}_{ti}")
                    nc.scalar.activation(ptile[:], st[:], mybir.ActivationFunctionType.Exp,
                                         scale=0.125)
                    pts[ti] = ptile
                    if ti > 0:
                        pv(h, ti - 1, pts[ti - 1])
                    if 0 <= ti - 1 < len(tail_cl):
                        tail_cl[ti - 1]()
                pv(h, TT - 1, pts[TT - 1])
                tail_copy(h)
            # head 7 tail normalize runs here
            for cl in emit_tail(H - 1):
                cl()

        # ---- final projection: out[s,d] = sum_h oT_hn[h]^T @ Wo_h ----
        with tc.tile_pool(name="fin_ps", bufs=2, space="PSUM") as fin_ps:
            fps = {}

            def fin_finish(sc):
                # h=7 contribution last: overlaps head-7's tail normalize
                nc.tensor.matmul(
                    fps[sc][:],
                    oT_hn[H - 1][0:P, sc * 128:(sc + 1) * 128],
                    wo_sb[0:P, (H - 1) * D:H * D],
                    start=False, stop=True,
                )
                fin = fin_pool.tile([128, 512], f32, tag="fin", name=f"fin{sc}")
                if sc % 2 == 0:
                    nc.vector.tensor_copy(fin[:], fps[sc][:])
                else:
                    nc.scalar.copy(fin[:], fps[sc][:])
                nc.sync.dma_start(out_d[sc * 128:(sc + 1) * 128, :], fin[:])

            for sc in range(QT):
                fp = fin_ps.tile([128, 512], f32, tag="fp", name=f"fp{sc}")
                for h in range(H - 1):
                    nc.tensor.matmul(
                        fp[:],
                        oT_hn[h][0:P, sc * 128:(sc + 1) * 128],
                        wo_sb[0:P, h * D:(h + 1) * D],
                        start=(h == 0), stop=False,
                    )
                fps[sc] = fp
                if sc >= 1:
                    fin_finish(sc - 1)
            fin_finish(QT - 1)

    nc.compile()
    return nc


def _host_prep(q, k, v, Wq, Wk, Wv, Wo):
    import ml_dtypes
    wq_a = np.ascontiguousarray(
        Wq.transpose(1, 0, 2).reshape(DCH, 128, H * P), dtype=np.float32)
    wk_a = np.ascontiguousarray(
        Wk.transpose(1, 0, 2).reshape(DCH, 128, H * P), dtype=np.float32)
    wv_a = np.ascontiguousarray(
        Wv.transpose(1, 0, 2).reshape(DCH, 128, H * P), dtype=np.float32)
    wo_a = np.ascontiguousarray(
        Wo.reshape(H, P, D).transpose(1, 0, 2).reshape(P, H * D)
    ).astype(ml_dtypes.bfloat16)
    in_maps = []
    for c in range(NCORES):
        b, half = c // 2, c % 2
        qT = np.ascontiguousarray(
            q[b, half * SC:(half + 1) * SC, :].T.reshape(DCH, 128, SC))
        kT = np.ascontiguousarray(k[b].T.reshape(DCH, 128, S2))
        vT = np.ascontiguousarray(v[b].T.reshape(DCH, 128, S2))
        in_maps.append({
            "qT": qT, "kT": kT, "vT": vT,
            "wq": wq_a, "wk": wk_a, "wv": wv_a, "wo": wo_a,
        })
    return in_maps


def kernel(q, k, v, Wq, Wk, Wv, Wo):
    nc = _build()
    from concourse.bass_utils import run_bass_kernel_spmd

    q = np.asarray(q, np.float32)
    k = np.asarray(k, np.float32)
    v = np.asarray(v, np.float32)
    in_maps = _host_prep(q, k, v, np.asarray(Wq, np.float32),
                         np.asarray(Wk, np.float32), np.asarray(Wv, np.float32),
                         np.asarray(Wo, np.float32))
    res = run_bass_kernel_spmd(nc, in_maps, core_ids=list(range(NCORES)))
    globals()["LAST_RES"] = res
    out = np.empty((B, S1, D), np.float32)
    for c, r in enumerate(res.results):
        b, half = c // 2, c % 2
        out[b, half * SC:(half + 1) * SC] = r["out"]
    return out


if __name__ == "__main__":
    rng = np.random.default_rng(0)
    qq = rng.standard_normal((B, S1, D), dtype=np.float32)
    kk = rng.standard_normal((B, S2, D), dtype=np.float32)
    vv = rng.standard_normal((B, S2, D), dtype=np.float32)
    wq = rng.standard_normal((H, D, P), dtype=np.float32)
    wk = rng.standard_normal((H, D, P), dtype=np.float32)
    wv = rng.standard_normal((H, D, P), dtype=np.float32)
    wo = rng.standard_normal((H * P, D), dtype=np.float32)
    o = kernel(qq, kk, vv, wq, wk, wv, wo)
    print("out", o.shape, o.dtype, np.abs(o).mean())
